# revision 44
# baseline (speedup 1.0000x reference)
"""GroupedQueryAttention (B=1, T=2048, D=4096, 32 q-heads / 8 kv-heads, hd=128)
on 8 trn2 NeuronCores.

Sharding: kv-head parallel for QKV+attention (core c owns kv head c and its
4 query heads), then sequence-parallel for the output projection. Two
pipelined AllToAlls redistribute y from head-sharded to T-sharded (8x less
wire than an AllGather): A2A#1 covers T columns [0,1024) and overlaps with
attention on [1024,2048); A2A#2 overlaps with the first half of the wo
matmul. Core d owns T columns [128d,128d+128) and [1024+128d,1024+128d+128).

All big matmuls run in bf16 (fp32 is 4 cycles/row on the PE, bf16 is 1);
softmax statistics stay in fp32 (partition_all_reduce on the idle GpSimd).
Softmax runs without max-subtraction: fp32 exp can't overflow at these
score magnitudes.
"""
import sys

sys.path.insert(0, "/opt/trn_rl_repo")

import numpy as np
import ml_dtypes

import concourse.bacc as bacc
import concourse.tile as tile
from concourse import bass_isa, mybir
from concourse.bass_utils import run_bass_kernel_spmd
from concourse.masks import make_identity

N_CORES = 8
T = 2048
DIM = 4096
HD = 128
NH = 32
NKV = 8
NREP = NH // NKV  # 4 query heads per core
NCHUNK = T // 512  # 4 chunks of 512 along T
NKT = DIM // 128  # 32 contraction tiles for the projections
F32 = mybir.dt.float32
BF16 = mybir.dt.bfloat16
NPBF16 = ml_dtypes.bfloat16
SCALE = 1.0 / float(np.sqrt(HD))

import os

MAXPHASE = int(os.environ.get("GQA_MAXPHASE", "4"))
NOL = bool(int(os.environ.get("GQA_NOL", "0")))  # timing diagnostic only
# Benchmarking aid: build the NEFF with the whole kernel body repeated k
# times; wall-time differencing between k builds isolates HW exec time.
ITERS = int(os.environ.get("GQA_ITERS", "1"))

_cached = {}


def _build_kernel():
    if "nc" in _cached:
        return _cached["nc"]

    nc = bacc.Bacc("TRN2", target_bir_lowering=False)

    xT = nc.dram_tensor("xT", [DIM, T], BF16, kind="ExternalInput")
    cos2 = nc.dram_tensor("cos2", [128, T], F32, kind="ExternalInput")
    sin2 = nc.dram_tensor("sin2", [128, T], F32, kind="ExternalInput")
    masks = nc.dram_tensor("masks", [128, 4 * 512], BF16, kind="ExternalInput")
    wqT = nc.dram_tensor("wqT", [DIM, NREP * HD], BF16, kind="ExternalInput")
    wkT = nc.dram_tensor("wkT", [DIM, HD], BF16, kind="ExternalInput")
    wvT = nc.dram_tensor("wvT", [DIM, HD], BF16, kind="ExternalInput")
    woT = nc.dram_tensor("woT", [DIM, DIM], BF16, kind="ExternalInput")
    out = nc.dram_tensor("out", [256, DIM], F32, kind="ExternalOutput")

    # AllToAll buffers, one pair per T-half. 8 row-blocks of 516: block d on
    # core c holds core c's 512 head-dims of UNNORMALIZED y for global column
    # block 128*(8*half+d)..+128, plus 4 rows of softmax denominators (one per
    # head, bf16). Normalization happens after the A2A, folded into phase 4.
    BLK = 512 + NREP  # 516
    y_in = [
        nc.dram_tensor(f"y_in{i}", [N_CORES * BLK, 128], BF16, kind="Internal")
        for i in range(2)
    ]
    y_out = [
        nc.dram_tensor(f"y_out{i}", [N_CORES * BLK, 128], BF16, kind="Internal")
        for i in range(2)
    ]

    with tile.TileContext(nc) as tc:
        with (
            tc.tile_pool(name="consts", bufs=1) as consts,
            tc.tile_pool(name="weights", bufs=1) as weights,
            tc.tile_pool(name="acts", bufs=1) as acts,
            tc.tile_pool(name="stream", bufs=1) as stream,
            tc.tile_pool(name="work", bufs=2) as work,
            tc.tile_pool(name="expp", bufs=6) as expp,
            tc.tile_pool(name="outp", bufs=3) as outp,
            tc.tile_pool(name="wop", bufs=2) as wop,
            tc.tile_pool(name="psum", bufs=8, space="PSUM") as psum,
        ):
            for _it in range(ITERS):
                # ---------- constants ----------
                cos_sb = consts.tile([128, T], F32, tag="cos")
                nc.sync.dma_start(out=cos_sb, in_=cos2[:, :])
                sin_sb = consts.tile([128, T], F32, tag="sin")
                nc.sync.dma_start(out=sin_sb, in_=sin2[:, :])
                mask_sb = consts.tile([128, 4 * 512], BF16, tag="mask")
                nc.sync.dma_start(out=mask_sb, in_=masks[:, :])
                ident = consts.tile([128, 128], BF16, tag="ident")
                make_identity(nc, ident)

                # ---------- resident weights (wq chunked so kt=0 is ready early) ----------
                wq_sb = []
                for i in range(4):
                    t = weights.tile([128, 8, NREP * HD], BF16, tag=f"wq{i}")
                    nc.sync.dma_start(
                        out=t,
                        in_=wqT.rearrange("(n p) m -> p n m", p=128)[:, 8 * i:8 * (i + 1), :],
                    )
                    wq_sb.append(t)
                wk_sb = weights.tile([128, NKT, HD], BF16, tag="wk")
                nc.sync.dma_start(
                    out=wk_sb, in_=wkT.rearrange("(n p) m -> p n m", p=128)
                )
                wv_sb = weights.tile([128, NKT, HD], BF16, tag="wv")
                nc.sync.dma_start(
                    out=wv_sb, in_=wvT.rearrange("(n p) m -> p n m", p=128)
                )

                # activations that live through the attention phase
                qT_sb = acts.tile([128, NREP, T], BF16, tag="qt")
                kT_sb = acts.tile([128, T], BF16, tag="kt")
                vkd_sb = acts.tile([128, T // 128, HD], BF16, tag="vkd")

                # ---------- phase 1: QKV projections + rope ----------
                # x is chunk-resident (two 2 MB DMAs per 512-col chunk); the
                # PSUM accumulation runs in 256-col half-chunks so the rope
                # tail of one half overlaps the matmuls of the next.
                for qc in range(NCHUNK):
                    x_ck = []
                    for i in range(2):
                        t = stream.tile([128, 16, 512], BF16, tag=f"x{i}")
                        nc.sync.dma_start(
                            out=t,
                            in_=xT[
                                2048 * i:2048 * (i + 1),
                                512 * qc:512 * (qc + 1),
                            ].rearrange("(n p) m -> p n m", p=128),
                        )
                        x_ck.append(t)
                    cs = slice(512 * qc, 512 * (qc + 1))
                    q_ps = [
                        psum.tile([128, 512], F32, tag="bank", name=f"qps{qc}_{h}")
                        for h in range(NREP)
                    ]
                    k_ps = psum.tile([128, 512], F32, tag="bank")
                    v_ps = psum.tile([128, 512], F32, tag="bank")
                    for kt in range(NKT):
                        xt = x_ck[kt // 16][:, kt % 16, :]
                        st = kt == 0
                        sp = kt == NKT - 1
                        for h in range(NREP):
                            nc.tensor.matmul(
                                q_ps[h],
                                lhsT=wq_sb[kt // 8][:, kt % 8, 128 * h:128 * (h + 1)],
                                rhs=xt,
                                start=st,
                                stop=sp,
                            )
                        nc.tensor.matmul(
                            k_ps, lhsT=wk_sb[:, kt, :], rhs=xt, start=st, stop=sp
                        )
                        nc.tensor.matmul(
                            v_ps, lhsT=wv_sb[:, kt, :], rhs=xt, start=st, stop=sp
                        )

                    # v computed in [hd, T] layout; transpose 128x128 blocks
                    v_sb = work.tile([128, 512], BF16, tag="vsb")
                    nc.scalar.copy(v_sb, v_ps)
                    for s in range(4):
                        vt_ps = psum.tile(
                            [128, 128], BF16, tag="bank", name=f"vt{qc}_{s}"
                        )
                        with nc.allow_low_precision(reason="pure transpose, no accumulation"):
                            nc.tensor.transpose(
                                vt_ps, v_sb[:, 128 * s:128 * (s + 1)], ident
                            )
                        nc.scalar.copy(vkd_sb[:, 4 * qc + s, :], vt_ps)

                    # rope for the 4 q heads and k (fp32 math, bf16 store)
                    for h in range(NREP + 1):
                        p = q_ps[h] if h < NREP else k_ps
                        dst = qT_sb[:, h, cs] if h < NREP else kT_sb[:, cs]
                        sw = work.tile([128, 512], F32, tag="sw")
                        nc.scalar.copy(sw[0:64, :], p[64:128, :])
                        nc.scalar.copy(sw[64:128, :], p[0:64, :])
                        d1 = work.tile([128, 512], F32, tag="d1")
                        nc.vector.tensor_mul(d1, p, cos_sb[:, cs])
                        nc.vector.tensor_mul(sw, sw, sin_sb[:, cs])
                        nc.vector.tensor_add(dst, d1, sw)

                # ---------- phase 2 + 3: attention, A2A#1 after first T-half ----------
                def attention_chunk(qc):
                    cs = slice(512 * qc, 512 * (qc + 1))
                    nkt = 4 * qc + 4  # causal: k tiles 0 .. 4*qc+3
                    for h in range(NREP):
                        yT_ps = psum.tile([128, 512], F32, tag="bank")
                        l_acc = work.tile([128, 512], F32, tag="lacc")
                        if not NOL:
                            nc.vector.memset(l_acc, 0.0)
                        e_pair = None
                        for kt in range(nkt):
                            d = kt - 4 * qc
                            # diagonal blocks: columns q < 128d are fully
                            # masked (k+128d<=q never holds) — skip them in
                            # score/exp/mask/l; zero them for the y matmul
                            lo = 128 * d if d > 0 else 0
                            qs = slice(512 * qc + lo, 512 * (qc + 1))
                            sT_ps = psum.tile([128, 512], F32, tag="bank")
                            nc.tensor.matmul(
                                sT_ps[:, lo:512],
                                lhsT=kT_sb[:, 128 * kt:128 * (kt + 1)],
                                rhs=qT_sb[:, h, qs],
                                start=True,
                                stop=True,
                            )
                            e_sb = expp.tile([128, 512], BF16, tag="exp")
                            if lo > 0:
                                nc.vector.memset(e_sb[:, 0:lo], 0.0)
                            nc.scalar.activation(
                                e_sb[:, lo:512], sT_ps[:, lo:512],
                                mybir.ActivationFunctionType.Exp,
                                scale=SCALE,
                            )
                            if d >= 0:  # diagonal block: zero the k > q half
                                nc.vector.tensor_mul(
                                    e_sb[:, lo:512], e_sb[:, lo:512],
                                    mask_sb[:, 512 * d + lo:512 * (d + 1)],
                                )
                            if not NOL:
                                # tree accumulation: pair-sum in bf16 (2x DVE
                                # rate), fold into fp32 l_acc every 2nd tile
                                if kt % 2 == 0:
                                    e_pair = e_sb
                                else:
                                    pr = work.tile([128, 512], BF16, tag="epair")
                                    nc.vector.tensor_add(pr, e_pair, e_sb)
                                    nc.vector.tensor_add(l_acc, l_acc, pr)
                            nc.tensor.matmul(
                                yT_ps,
                                lhsT=vkd_sb[:, kt, :],
                                rhs=e_sb,
                                start=(kt == 0),
                                stop=(kt == nkt - 1),
                            )
                        # y ships UNNORMALIZED; the softmax denominator is
                        # partition-summed on the idle GpSimd (off the PE/DVE
                        # critical path) and rides along in the A2A payload
                        yn_sb = outp.tile([128, 512], BF16, tag="yn")
                        nc.vector.tensor_copy(yn_sb, yT_ps)
                        if not NOL:
                            l_red = work.tile([128, 512], F32, tag="lred")
                            nc.gpsimd.partition_all_reduce(
                                l_red, l_acc, channels=128,
                                reduce_op=bass_isa.ReduceOp.add,
                            )
                            l_bf = work.tile([1, 512], BF16, tag="lbf")
                            nc.vector.tensor_copy(l_bf, l_red[0:1, :])
                        # scatter the four 128-col blocks to their A2A slots
                        for s in range(4):
                            b = 4 * qc + s  # global 128-col block index
                            half, dst_core = b // 8, b % 8
                            nc.sync.dma_start(
                                out=y_in[half][
                                    BLK * dst_core + 128 * h:
                                    BLK * dst_core + 128 * (h + 1), :
                                ],
                                in_=yn_sb[:, 128 * s:128 * (s + 1)],
                            )
                            if not NOL:
                                nc.sync.dma_start(
                                    out=y_in[half][
                                        BLK * dst_core + 512 + h:
                                        BLK * dst_core + 512 + h + 1, :
                                    ],
                                    in_=l_bf[0:1, 128 * s:128 * (s + 1)],
                                )

                def a2a(half):
                    nc.gpsimd.collective_compute(
                        "AllToAll",
                        mybir.AluOpType.bypass,
                        ins=[y_in[half][:, :]],
                        outs=[y_out[half][:, :]],
                        replica_groups=[list(range(N_CORES))],
                    )

                if MAXPHASE >= 2:
                    attention_chunk(0)
                    attention_chunk(1)
                    if MAXPHASE >= 3:
                        a2a(0)
                    attention_chunk(2)
                    attention_chunk(3)
                    if MAXPHASE >= 3:
                        a2a(1)

                # ---------- phase 4: out = y @ wo.T, single pass over wo ----------
                if MAXPHASE >= 4:
                    yh_sb = []
                    for half in range(2):
                        # 4 tiles of 8 kt-slices each so the normalize of one
                        # tile pipelines with matmuls on already-done tiles
                        ts = [
                            weights.tile([128, 8, 128], BF16, tag=f"ysb{half}_{i}")
                            for i in range(4)
                        ]
                        l_sb = consts.tile([1, NKT * 128], BF16, tag="lsb")
                        for c in range(N_CORES):
                            nc.sync.dma_start(
                                out=ts[c // 2][:, 4 * (c % 2):4 * (c % 2 + 1), :],
                                in_=y_out[half][
                                    BLK * c:BLK * c + 512, :
                                ].rearrange("(n p) m -> p n m", p=128),
                            )
                            if not NOL:
                                # 4 head-rows of l land flat at cols 512c..
                                nc.sync.dma_start(
                                    out=l_sb[0:1, 512 * c:512 * (c + 1)],
                                    in_=y_out[half][BLK * c + 512:BLK * c + 516, :],
                                )
                        if not NOL:
                            # normalize: column q of kt-tile kt scales by
                            # 1/l[kt][q]; flat offset of tile kt's l is 128*kt
                            for kt in range(NKT):
                                lr = work.tile([1, 128], F32, tag="lr")
                                nc.vector.reciprocal(
                                    lr, l_sb[0:1, 128 * kt:128 * (kt + 1)]
                                )
                                rb = work.tile([128, 128], F32, tag="rb")
                                nc.gpsimd.partition_broadcast(rb, lr[0:1, :])
                                nc.vector.tensor_mul(
                                    ts[kt // 8][:, kt % 8, :],
                                    ts[kt // 8][:, kt % 8, :], rb
                                )
                        yh_sb.append(ts)
                    for g in range(4):  # four groups of 2 output column slices
                        o_ps = [
                            psum.tile([128, 512], F32, tag="bank", name=f"o{g}_{hc}")
                            for hc in range(4)  # 2 row-halves x 2 col slices
                        ]
                        for kt4 in range(NKT // 4):
                            # 1 MB DMA (4 kt-rows x 1024 cols): one InstDMACopy
                            # fans out over all 16 SDMA engines, ~peak HBM BW
                            wo_t = wop.tile([128, 4, 2, 512], BF16, tag="wot")
                            nc.sync.dma_start(
                                out=wo_t,
                                in_=woT[
                                    512 * kt4:512 * (kt4 + 1),
                                    1024 * g:1024 * (g + 1),
                                ].rearrange("(n p) (c m) -> p n c m", p=128, c=2),
                            )
                            for kn in range(4):
                                kt = 4 * kt4 + kn
                                for half in range(2):
                                    for c in range(2):
                                        nc.tensor.matmul(
                                            o_ps[2 * half + c],
                                            lhsT=yh_sb[half][:, kt, :],
                                            rhs=wo_t[:, kn, c, :],
                                            start=(kt == 0),
                                            stop=(kt == NKT - 1),
                                        )
                        for half in range(2):
                            for c in range(2):
                                o_sb = outp.tile([128, 512], F32, tag="osb")
                                nc.scalar.copy(o_sb, o_ps[2 * half + c])
                                nc.sync.dma_start(
                                    out=out[
                                        128 * half:128 * (half + 1),
                                        1024 * g + 512 * c:1024 * g + 512 * (c + 1),
                                    ],
                                    in_=o_sb,
                                )

    nc.compile()
    _cached["nc"] = nc
    return nc


def _build_in_maps(inputs):
    return _shard_inputs(**inputs)


def _shard_inputs(x, cos, sin, wq, wk, wv, wo, start_pos):
    x = np.asarray(x, dtype=np.float32)
    cos = np.asarray(cos, dtype=np.float32)
    sin = np.asarray(sin, dtype=np.float32)
    wq = np.asarray(wq, dtype=np.float32)
    wk = np.asarray(wk, dtype=np.float32)
    wv = np.asarray(wv, dtype=np.float32)
    wo = np.asarray(wo, dtype=np.float32)
    sp = int(start_pos)

    xT = np.ascontiguousarray(x[0].T).astype(NPBF16)  # (DIM, T)
    cosT = np.ascontiguousarray(cos[sp:sp + T].T)  # (64, T)
    sinT = np.ascontiguousarray(sin[sp:sp + T].T)
    cos2 = np.concatenate([cosT, cosT], axis=0)  # (128, T)
    sin2 = np.concatenate([-sinT, sinT], axis=0)  # rotate-half signs folded in

    kk = np.arange(128)[:, None]
    qq = np.arange(512)[None, :]
    masks = np.concatenate(
        [(kk + 128 * d <= qq).astype(NPBF16) for d in range(4)], axis=1
    )  # (128, 2048)

    woT = np.ascontiguousarray(wo.T).astype(NPBF16)  # (DIM, DIM), full

    in_maps = []
    for c in range(N_CORES):
        qrows = slice(NREP * HD * c, NREP * HD * (c + 1))
        krows = slice(HD * c, HD * (c + 1))
        in_maps.append({
            "xT": xT,
            "cos2": cos2,
            "sin2": sin2,
            "masks": masks,
            "wqT": np.ascontiguousarray(wq[qrows, :].T).astype(NPBF16),
            "wkT": np.ascontiguousarray(wk[krows, :].T).astype(NPBF16),
            "wvT": np.ascontiguousarray(wv[krows, :].T).astype(NPBF16),
            "woT": woT,
        })
    return in_maps


def kernel(x, cos, sin, wq, wk, wv, wo, start_pos):
    in_maps = _shard_inputs(x, cos, sin, wq, wk, wv, wo, start_pos)
    nc = _build_kernel()
    res = run_bass_kernel_spmd(nc, in_maps, core_ids=list(range(N_CORES)))
    # core d returns T rows [128d,128d+128) and [1024+128d,1024+128d+128)
    full = np.empty((T, DIM), np.float32)
    for d in range(N_CORES):
        o = res.results[d]["out"]
        full[128 * d:128 * (d + 1)] = o[0:128]
        full[1024 + 128 * d:1024 + 128 * (d + 1)] = o[128:256]
    return full.reshape(1, T, DIM)


# revision 65
# speedup vs baseline: 1.1093x; 1.1093x over previous
"""GroupedQueryAttention (B=1, T=2048, D=4096, 32 q-heads / 8 kv-heads, hd=128)
on 8 trn2 NeuronCores.

Sharding: kv-head parallel for QKV+attention (core c owns kv head c and its
4 query heads), then sequence-parallel for the output projection. Two
pipelined AllToAlls redistribute y from head-sharded to T-sharded (8x less
wire than an AllGather): A2A#1 covers T columns [0,1024) and overlaps with
attention on [1024,2048); A2A#2 overlaps with the first half of the wo
matmul. Core d owns T columns [128d,128d+128) and [1024+128d,1024+128d+128).

All big matmuls run in bf16 (fp32 is 4 cycles/row on the PE, bf16 is 1);
softmax statistics stay in fp32 (partition_all_reduce on the idle GpSimd).
Softmax runs without max-subtraction: fp32 exp can't overflow at these
score magnitudes.
"""
import sys

sys.path.insert(0, "/opt/trn_rl_repo")

import numpy as np
import ml_dtypes

import concourse.bacc as bacc
import concourse.tile as tile
from concourse import bass_isa, mybir
from concourse.bass_utils import run_bass_kernel_spmd
from concourse.masks import make_identity

N_CORES = 8
T = 2048
DIM = 4096
HD = 128
NH = 32
NKV = 8
NREP = NH // NKV  # 4 query heads per core
NCHUNK = T // 512  # 4 chunks of 512 along T
NKT = DIM // 128  # 32 contraction tiles for the projections
F32 = mybir.dt.float32
BF16 = mybir.dt.bfloat16
NPBF16 = ml_dtypes.bfloat16
SCALE = 1.0 / float(np.sqrt(HD))

import os

MAXPHASE = int(os.environ.get("GQA_MAXPHASE", "4"))
NOL = bool(int(os.environ.get("GQA_NOL", "0")))  # timing diagnostic only
# Benchmarking aid: build the NEFF with the whole kernel body repeated k
# times; wall-time differencing between k builds isolates HW exec time.
ITERS = int(os.environ.get("GQA_ITERS", "1"))

_cached = {}


def _build_kernel():
    if "nc" in _cached:
        return _cached["nc"]

    nc = bacc.Bacc("TRN2", target_bir_lowering=False)

    xT = nc.dram_tensor("xT", [DIM, T], BF16, kind="ExternalInput")
    cos2 = nc.dram_tensor("cos2", [128, T], F32, kind="ExternalInput")
    sin2 = nc.dram_tensor("sin2", [128, T], F32, kind="ExternalInput")
    masks = nc.dram_tensor("masks", [128, 4 * 512], BF16, kind="ExternalInput")
    wqT = nc.dram_tensor("wqT", [DIM, NREP * HD], BF16, kind="ExternalInput")
    wkT = nc.dram_tensor("wkT", [DIM, HD], BF16, kind="ExternalInput")
    wvT = nc.dram_tensor("wvT", [DIM, HD], BF16, kind="ExternalInput")
    woT = nc.dram_tensor("woT", [DIM, DIM], BF16, kind="ExternalInput")
    out = nc.dram_tensor("out", [256, DIM], F32, kind="ExternalOutput")

    # AllToAll buffers, one pair per T-half. 8 row-blocks of 516: block d on
    # core c holds core c's 512 head-dims of UNNORMALIZED y for global column
    # block 128*(8*half+d)..+128, plus 4 rows of softmax denominators (one per
    # head, bf16). Normalization happens after the A2A, folded into phase 4.
    BLK = 512 + NREP  # 516
    y_in = [
        nc.dram_tensor(f"y_in{i}", [N_CORES * BLK, 128], BF16, kind="Internal")
        for i in range(2)
    ]
    y_out = [
        nc.dram_tensor(f"y_out{i}", [N_CORES * BLK, 128], BF16, kind="Internal")
        for i in range(2)
    ]

    with tile.TileContext(nc) as tc:
        with (
            tc.tile_pool(name="consts", bufs=1) as consts,
            tc.tile_pool(name="weights", bufs=1) as weights,
            tc.tile_pool(name="acts", bufs=1) as acts,
            tc.tile_pool(name="stream", bufs=1) as stream,
            tc.tile_pool(name="work", bufs=2) as work,
            tc.tile_pool(name="expp", bufs=2) as expp,
            tc.tile_pool(name="outp", bufs=2) as outp,
            tc.tile_pool(name="wop", bufs=2) as wop,
            tc.tile_pool(name="psum", bufs=8, space="PSUM") as psum,
        ):
            for _it in range(ITERS):
                def load_x(qc):
                    x_ck = []
                    for i in range(4):
                        t = stream.tile(
                            [128, 8, 512], BF16, tag=f"x{i % 2}", name=f"x{qc}_{i}"
                        )
                        nc.sync.dma_start(
                            out=t,
                            in_=xT[
                                1024 * i:1024 * (i + 1),
                                512 * qc:512 * (qc + 1),
                            ].rearrange("(n p) m -> p n m", p=128),
                        )
                        x_ck.append(t)
                    return x_ck

                # ---------- resident weights, in first-use order so the
                # first chunk's matmuls (k/v first) start as early as possible
                wk_sb = weights.tile([128, NKT, HD], BF16, tag="wk")
                nc.sync.dma_start(
                    out=wk_sb, in_=wkT.rearrange("(n p) m -> p n m", p=128)
                )
                wv_sb = weights.tile([128, NKT, HD], BF16, tag="wv")
                nc.sync.dma_start(
                    out=wv_sb, in_=wvT.rearrange("(n p) m -> p n m", p=128)
                )
                x_pre = load_x(0)
                wq_sb = []
                for i in range(4):
                    t = weights.tile([128, 8, NREP * HD], BF16, tag=f"wq{i}")
                    nc.sync.dma_start(
                        out=t,
                        in_=wqT.rearrange("(n p) m -> p n m", p=128)[:, 8 * i:8 * (i + 1), :],
                    )
                    wq_sb.append(t)

                # constants: not needed until the first rope, ~40us in
                cos_sb = consts.tile([128, T], F32, tag="cos")
                nc.sync.dma_start(out=cos_sb, in_=cos2[:, :])
                sin_sb = consts.tile([128, T], F32, tag="sin")
                nc.sync.dma_start(out=sin_sb, in_=sin2[:, :])
                mask_sb = consts.tile([128, 4, 512], BF16, tag="mask")
                nc.sync.dma_start(
                    out=mask_sb, in_=masks.rearrange("p (d m) -> p d m", d=4)
                )
                ident = consts.tile([128, 128], BF16, tag="ident")
                make_identity(nc, ident)

                # activations that live through the attention phase —
                # PER-CHUNK tiles so attention chunk qc only depends on the
                # phase-1 chunks it actually reads (Tile deps are per-tile)
                qT_t = [
                    acts.tile([128, NREP, 512], BF16, tag=f"qt{i}", name=f"qt{i}")
                    for i in range(NCHUNK)
                ]
                kT_t = [
                    acts.tile([128, 512], BF16, tag=f"kt{i}", name=f"kt{i}")
                    for i in range(NCHUNK)
                ]
                vkd_t = [
                    acts.tile([128, 4, HD], BF16, tag=f"vkd{i}", name=f"vkd{i}")
                    for i in range(NCHUNK)
                ]

                # ---------- phase 1: QKV projections + rope ----------
                # x is chunk-resident (two 2 MB DMAs per 512-col chunk)
                def p1_chunk(qc):
                    x_ck = x_pre if qc == 0 else load_x(qc)
                    cs = slice(512 * qc, 512 * (qc + 1))
                    q_ps = [
                        psum.tile([128, 512], F32, tag="bank", name=f"qps{qc}_{h}")
                        for h in range(NREP)
                    ]
                    k_ps = psum.tile([128, 512], F32, tag="bank")
                    v_ps = psum.tile([128, 512], F32, tag="bank")
                    for kt in range(NKT):
                        xt = x_ck[kt // 8][:, kt % 8, :]
                        st = kt == 0
                        sp = kt == NKT - 1
                        nc.tensor.matmul(
                            k_ps, lhsT=wk_sb[:, kt, :], rhs=xt, start=st, stop=sp
                        )
                        nc.tensor.matmul(
                            v_ps, lhsT=wv_sb[:, kt, :], rhs=xt, start=st, stop=sp
                        )
                        for h in range(NREP):
                            nc.tensor.matmul(
                                q_ps[h],
                                lhsT=wq_sb[kt // 8][:, kt % 8, 128 * h:128 * (h + 1)],
                                rhs=xt,
                                start=st,
                                stop=sp,
                            )

                    # v computed in [hd, T] layout; transpose 128x128 blocks
                    v_sb = work.tile([128, 512], BF16, tag="vsb")
                    nc.scalar.copy(v_sb, v_ps)
                    for s in range(4):
                        vt_ps = psum.tile(
                            [128, 128], BF16, tag="bank", name=f"vt{qc}_{s}"
                        )
                        with nc.allow_low_precision(reason="pure transpose, no accumulation"):
                            nc.tensor.transpose(
                                vt_ps, v_sb[:, 128 * s:128 * (s + 1)], ident
                            )
                        nc.scalar.copy(vkd_t[qc][:, s, :], vt_ps)

                    # rope for the 4 q heads and k (fp32 math, bf16 store)
                    for h in range(NREP + 1):
                        p = q_ps[h] if h < NREP else k_ps
                        dst = qT_t[qc][:, h, :] if h < NREP else kT_t[qc][:, :]
                        sw = work.tile([128, 512], F32, tag="sw")
                        nc.scalar.copy(sw[0:64, :], p[64:128, :])
                        nc.scalar.copy(sw[64:128, :], p[0:64, :])
                        nc.vector.tensor_mul(sw, sw, sin_sb[:, cs])
                        d1 = work.tile([128, 512], F32, tag="lred")
                        nc.vector.tensor_mul(d1, p, cos_sb[:, cs])
                        nc.vector.tensor_add(dst, d1, sw)

                # ---------- phase 2 + 3: attention, A2A#1 after first T-half ----------
                def attention_chunk(qc):
                    cs = slice(512 * qc, 512 * (qc + 1))
                    nkt = 4 * qc + 4  # causal: k tiles 0 .. 4*qc+3
                    for h in range(NREP):
                        yT_ps = psum.tile([128, 512], F32, tag="bank")
                        # all e tiles of this (h,qc) live in one contiguous
                        # tile: one batched diagonal mask, log-tree l-reduce
                        e_full = expp.tile(
                            [128, 16, 512], BF16, tag="efull", name=f"ef{qc}_{h}"
                        )
                        for kt in range(nkt):
                            d = kt - 4 * qc
                            # diagonal blocks: columns q < 128d are fully
                            # masked (k+128d<=q never holds) — skip them in
                            # score/exp; zero them for the y matmul
                            lo = 128 * d if d > 0 else 0
                            qs = slice(512 * qc + lo, 512 * (qc + 1))
                            sT_ps = psum.tile([128, 512], F32, tag="bank")
                            nc.tensor.matmul(
                                sT_ps[:, lo:512],
                                lhsT=kT_t[kt // 4][:, 128 * (kt % 4):128 * (kt % 4 + 1)],
                                rhs=qT_t[qc][:, h, lo:512],
                                start=True,
                                stop=True,
                            )
                            if lo > 0:
                                # prefix is masked to zero later, but must be
                                # finite: first-touch SBUF can hold NaN bits
                                nc.vector.memset(e_full[:, kt, 0:lo], 0.0)
                            nc.scalar.activation(
                                e_full[:, kt, lo:512], sT_ps[:, lo:512],
                                mybir.ActivationFunctionType.Exp,
                                scale=SCALE,
                            )
                            if kt < nkt - 4:  # non-diagonal: e is final
                                nc.tensor.matmul(
                                    yT_ps,
                                    lhsT=vkd_t[kt // 4][:, kt % 4, :],
                                    rhs=e_full[:, kt, :],
                                    start=(kt == 0),
                                    stop=False,
                                )
                        # batched mask for the 4 diagonal tiles (last 4 kts)
                        nc.vector.tensor_mul(
                            e_full[:, nkt - 4:nkt, :], e_full[:, nkt - 4:nkt, :],
                            mask_sb,
                        )
                        for kt in range(nkt - 4, nkt):
                            d = kt - 4 * qc
                            lo = 128 * d if d > 0 else 0
                            nc.tensor.matmul(
                                yT_ps[:, lo:512],
                                lhsT=vkd_t[kt // 4][:, kt % 4, :],
                                rhs=e_full[:, kt, lo:512],
                                start=(kt == 0),
                                stop=(kt == nkt - 1),
                            )
                        # y ships UNNORMALIZED; the softmax denominator is
                        # partition-summed on the idle GpSimd (off the PE/DVE
                        # critical path) and rides along in the A2A payload
                        yn_sb = outp.tile([128, 512], BF16, tag="yn")
                        nc.scalar.copy(yn_sb, yT_ps)
                        if not NOL:
                            # in-place log-tree sum over the nkt tiles (bf16,
                            # 2x DVE rate; runs after the y matmuls read e)
                            w = nkt
                            while w > 1:
                                h2 = w // 2
                                nc.vector.tensor_add(
                                    e_full[:, 0:h2, :], e_full[:, 0:h2, :],
                                    e_full[:, h2:2 * h2, :],
                                )
                                if w % 2 == 1:
                                    nc.vector.tensor_add(
                                        e_full[:, 0:1, :], e_full[:, 0:1, :],
                                        e_full[:, w - 1:w, :],
                                    )
                                w = h2
                            l_acc = work.tile([128, 512], F32, tag="lacc")
                            nc.vector.tensor_copy(l_acc, e_full[:, 0, :])
                            l_red = work.tile([128, 512], F32, tag="lred")
                            nc.gpsimd.partition_all_reduce(
                                l_red, l_acc, channels=128,
                                reduce_op=bass_isa.ReduceOp.add,
                            )
                            l_bf = work.tile([1, 512], BF16, tag="lbf")
                            nc.vector.tensor_copy(l_bf, l_red[0:1, :])
                        # scatter to the A2A buffer: the 4 dest blocks are
                        # at regular BLK strides -> ONE strided DMA for y and
                        # one for l (DMA-issue cost on SP is ~1.3us each)
                        half, dst0 = qc // 2, 4 * (qc % 2)
                        yv = y_in[half].rearrange("(d r) m -> r d m", r=BLK)
                        nc.sync.dma_start(
                            out=yv[128 * h:128 * (h + 1), dst0:dst0 + 4, :],
                            in_=yn_sb,
                        )
                        if not NOL:
                            nc.sync.dma_start(
                                out=yv[512 + h:512 + h + 1, dst0:dst0 + 4, :],
                                in_=l_bf,
                            )

                def a2a(half):
                    nc.gpsimd.collective_compute(
                        "AllToAll",
                        mybir.AluOpType.bypass,
                        ins=[y_in[half][:, :]],
                        outs=[y_out[half][:, :]],
                        replica_groups=[list(range(N_CORES))],
                    )

                def load_norm(half):
                    # y load + softmax normalization for one out-row half;
                    # runs right after its A2A, overlapping later work.
                    # 4 tiles of 8 kt-slices so normalize pipelines.
                    ts = [
                        weights.tile(
                            [128, 8, 128], BF16, tag=f"ysb{half}_{i}",
                            name=f"yh{half}_{i}",
                        )
                        for i in range(4)
                    ]
                    l_sb = consts.tile(
                        [1, NKT * 128], BF16, tag="lsb", name=f"lsb{half}"
                    )
                    yv = y_out[half].rearrange("(d r) m -> d r m", r=BLK)
                    for i in range(4):
                        for j in range(2):
                            nc.sync.dma_start(
                                out=ts[i][:, 4 * j:4 * (j + 1), :],
                                in_=yv[2 * i + j, 0:512, :].rearrange(
                                    "(n p) m -> p n m", p=128
                                ),
                            )
                    # all 32 head-rows of l in one strided DMA, flat order
                    # (block, head, col) == l_sb column order 512c+128h+m
                    nc.sync.dma_start(out=l_sb, in_=yv[:, 512:516, :])
                    # normalize: column q of kt-tile kt scales by 1/l[kt][q];
                    # flat offset of tile kt's l is 128*kt
                    for kt in range(NKT):
                        lr = work.tile([1, 128], F32, tag="lr")
                        nc.vector.reciprocal(
                            lr, l_sb[0:1, 128 * kt:128 * (kt + 1)]
                        )
                        rb = work.tile([128, 128], F32, tag="rb")
                        nc.gpsimd.partition_broadcast(rb, lr[0:1, :])
                        nc.vector.tensor_mul(
                            ts[kt // 8][:, kt % 8, :],
                            ts[kt // 8][:, kt % 8, :], rb
                        )
                    return ts

                # interleave: attention chunk qc only needs phase-1
                # chunks 0..qc, and placing it early in program order lets
                # it claim PSUM banks / DVE / ACT while later phase-1
                # chunks keep the PE busy
                p1_chunk(0)
                p1_chunk(1)
                if MAXPHASE >= 2:
                    attention_chunk(0)
                p1_chunk(2)
                if MAXPHASE >= 2:
                    attention_chunk(1)
                    if MAXPHASE >= 3:
                        a2a(0)
                p1_chunk(3)
                if MAXPHASE >= 2:
                    attention_chunk(2)
                    yh0 = load_norm(0) if MAXPHASE >= 4 else None
                    attention_chunk(3)
                    if MAXPHASE >= 3:
                        a2a(1)
                    yh1 = load_norm(1) if MAXPHASE >= 4 else None

                # ---------- phase 4: out = y @ wo.T, single pass over wo ----------
                if MAXPHASE >= 4:
                    yh_sb = [yh0, yh1]
                    for g in range(4):  # four groups of 2 col slices x 2 halves
                        o_ps = [
                            psum.tile(
                                [128, 512], F32, tag="bank", name=f"o{g}_{hc}"
                            )
                            for hc in range(4)
                        ]
                        for kt4 in range(NKT // 4):
                            # 1 MB DMA: one InstDMACopy fans out over all 16
                            # SDMA engines, ~peak HBM BW
                            wo_t = wop.tile([128, 4, 2, 512], BF16, tag="wot")
                            nc.sync.dma_start(
                                out=wo_t,
                                in_=woT[
                                    512 * kt4:512 * (kt4 + 1),
                                    1024 * g:1024 * (g + 1),
                                ].rearrange("(n p) (c m) -> p n c m", p=128, c=2),
                            )
                            for kn in range(4):
                                kt = 4 * kt4 + kn
                                for half in range(2):
                                    for c in range(2):
                                        nc.tensor.matmul(
                                            o_ps[2 * half + c],
                                            lhsT=yh_sb[half][kt // 8][:, kt % 8, :],
                                            rhs=wo_t[:, kn, c, :],
                                            start=(kt == 0),
                                            stop=(kt == NKT - 1),
                                        )
                        for half in range(2):
                            for c in range(2):
                                o_sb = outp.tile([128, 512], F32, tag="osb")
                                nc.scalar.copy(o_sb, o_ps[2 * half + c])
                                nc.sync.dma_start(
                                    out=out[
                                        128 * half:128 * (half + 1),
                                        1024 * g + 512 * c:1024 * g + 512 * (c + 1),
                                    ],
                                    in_=o_sb,
                                )

    nc.compile()
    _cached["nc"] = nc
    return nc


def _build_in_maps(inputs):
    return _shard_inputs(**inputs)


def _shard_inputs(x, cos, sin, wq, wk, wv, wo, start_pos):
    x = np.asarray(x, dtype=np.float32)
    cos = np.asarray(cos, dtype=np.float32)
    sin = np.asarray(sin, dtype=np.float32)
    wq = np.asarray(wq, dtype=np.float32)
    wk = np.asarray(wk, dtype=np.float32)
    wv = np.asarray(wv, dtype=np.float32)
    wo = np.asarray(wo, dtype=np.float32)
    sp = int(start_pos)

    xT = np.ascontiguousarray(x[0].T).astype(NPBF16)  # (DIM, T)
    cosT = np.ascontiguousarray(cos[sp:sp + T].T)  # (64, T)
    sinT = np.ascontiguousarray(sin[sp:sp + T].T)
    cos2 = np.concatenate([cosT, cosT], axis=0)  # (128, T)
    sin2 = np.concatenate([-sinT, sinT], axis=0)  # rotate-half signs folded in

    kk = np.arange(128)[:, None]
    qq = np.arange(512)[None, :]
    masks = np.concatenate(
        [(kk + 128 * d <= qq).astype(NPBF16) for d in range(4)], axis=1
    )  # (128, 2048)

    woT = np.ascontiguousarray(wo.T).astype(NPBF16)  # (DIM, DIM), full

    in_maps = []
    for c in range(N_CORES):
        qrows = slice(NREP * HD * c, NREP * HD * (c + 1))
        krows = slice(HD * c, HD * (c + 1))
        in_maps.append({
            "xT": xT,
            "cos2": cos2,
            "sin2": sin2,
            "masks": masks,
            "wqT": np.ascontiguousarray(wq[qrows, :].T).astype(NPBF16),
            "wkT": np.ascontiguousarray(wk[krows, :].T).astype(NPBF16),
            "wvT": np.ascontiguousarray(wv[krows, :].T).astype(NPBF16),
            "woT": woT,
        })
    return in_maps


def kernel(x, cos, sin, wq, wk, wv, wo, start_pos):
    in_maps = _shard_inputs(x, cos, sin, wq, wk, wv, wo, start_pos)
    nc = _build_kernel()
    res = run_bass_kernel_spmd(nc, in_maps, core_ids=list(range(N_CORES)))
    # core d returns T rows [128d,128d+128) and [1024+128d,1024+128d+128)
    full = np.empty((T, DIM), np.float32)
    for d in range(N_CORES):
        o = res.results[d]["out"]
        full[128 * d:128 * (d + 1)] = o[0:128]
        full[1024 + 128 * d:1024 + 128 * (d + 1)] = o[128:256]
    return full.reshape(1, T, DIM)
